# revision 9
# baseline (speedup 1.0000x reference)
"""AffineCoupling TRN2 kernel (v6).

Computes, for z [4_000_000, 16] fp32:
    zl = z[:, :8]; zr = z[:, 8:]
    log_s = MLP_logs(zl); b = MLP_b(zl)        (5 layers, LeakyReLU(0.01) between)
    out = concat([zl, yr]), yr = exp(log_s) * zr + b

Strategy (pure data parallel over 8 NeuronCores, ~508k rows each):
 - Contiguous DMA: core slice split into 31 macros of 16384 rows. natbf
   [128, 2048] bf16 holds 128 rows/partition (nat[p, c*16+f] = row p*128+c),
   loaded by ONE SWDGE cast-DMA (fp32 HBM -> bf16 SBUF, 8KB contiguous HBM
   per partition) and stored back by one SWDGE cast-DMA (bf16 -> fp32).
   The whole pipeline is bf16 (zl passthrough in bf16: ~1.3e-3 rel err,
   tolerance is 2e-2).
 - fwdT: 16 PE transpose-mode ops [128,128] -> x0ps bf16 PSUM (feature-major
   X layout: partition g*16+f, 8 groups of 16 feats); DVE 2x copy -> x0.
 - MLP: both branches fused in 16-wide groups (block-diagonal bf16 lhsT,
   same wmat as before); per layer 4 MMs N=512 -> h fp32 PSUM [128, 2048],
   one ACT Prelu (bias via per-partition operand) -> bf16 SBUF.
 - L5 -> hp5 [128, 2048] fp32 (e at partitions 0:64 as g*8+o, b at 64:128);
   ACT Exp (+bias) and DVE tensor_scalar_add assemble eb bf16.
 - backT: 16 transpose-mode ops -> ebT bf16 PSUM; combine in place:
   natbf_zr = e*zr + b via 2x (mul into tmp, add back), u-batched APs.
 - PSUM: x0ps(2) + h(4) + ebT(2) = 8 banks, single-buffered per tag;
   cross-macro overlap comes from fwdT/backT of adjacent macros.
"""
import os
import sys

sys.path.insert(0, "/opt/trn_rl_repo")
if "/root/.axon_site/_ro/trn_rl_repo" not in sys.path:
    sys.path.append("/root/.axon_site/_ro/trn_rl_repo")

import numpy as np

import concourse.bacc as bacc
import concourse.bass as bass
import concourse.tile as tile
from concourse import mybir
from concourse.bass import _add_dep_helper
from concourse.bass_utils import run_bass_kernel_spmd

FP = mybir.dt.float32
BF = mybir.dt.bfloat16

N_CORES = 8
BATCH = 4_000_000
ROWS_PER_MACRO = 16_384            # [128, 2048] bf16 nat tile, 128 rows/part
MACROS = 31
R = ROWS_PER_MACRO * MACROS        # 507,904 rows per core
PAD_ROWS = ROWS_PER_MACRO
CHUNKS = 4                         # 4096 rows each

STEP = 498_688
STARTS = [c * STEP for c in range(N_CORES - 1)] + [BATCH - R]

C_BIAS = 128
C_TOTAL = 135

LAST_RESULTS = None


def _build_consts(ws_logs, bs_logs, ws_b, bs_b):
    import ml_dtypes

    ws_logs = [np.asarray(w, np.float32) for w in ws_logs]
    bs_logs = [np.asarray(b, np.float32) for b in bs_logs]
    ws_b = [np.asarray(w, np.float32) for w in ws_b]
    bs_b = [np.asarray(b, np.float32) for b in bs_b]

    consts = np.zeros((128, C_TOTAL), np.float32)
    consts[:, 0:128] = np.eye(128, dtype=np.float32)
    for k in range(4):
        cat = np.concatenate([bs_logs[k], bs_b[k]])    # [16]
        consts[:, C_BIAS + k] = np.tile(cat, 8)
    consts[:, C_BIAS + 4] = np.concatenate(
        [np.tile(bs_logs[4], 8), np.tile(bs_b[4], 8)]
    )
    consts[:, C_BIAS + 5] = np.tile(bs_logs[4], 16)
    consts[:, C_BIAS + 6] = np.tile(bs_b[4], 16)

    # bf16 stationary matrices, lhsT k at cols [k*128, (k+1)*128)
    wmat = np.zeros((128, 5 * 128), np.float32)
    w1cat = np.vstack([ws_logs[0], ws_b[0]])           # [16, 8]
    for g in range(8):
        wmat[g * 16:g * 16 + 8, g * 16:(g + 1) * 16] = w1cat.T
    for k in (1, 2, 3):
        wk = np.zeros((16, 16), np.float32)
        wk[0:8, 0:8] = ws_logs[k]
        wk[8:16, 8:16] = ws_b[k]
        for g in range(8):
            wmat[g * 16:(g + 1) * 16, k * 128 + g * 16:k * 128 + (g + 1) * 16] = wk.T
    for g in range(8):
        wmat[g * 16:g * 16 + 8, 4 * 128 + g * 8:4 * 128 + (g + 1) * 8] = ws_logs[4].T
        wmat[g * 16 + 8:(g + 1) * 16,
             4 * 128 + 64 + g * 8:4 * 128 + 64 + (g + 1) * 8] = ws_b[4].T
    wmat = np.concatenate([wmat, np.eye(128, dtype=np.float32)], axis=1)
    wmat_bf = wmat.astype(ml_dtypes.bfloat16)
    return consts, wmat_bf


def _ap(t, offset, dims):
    return bass.AP(tensor=t.tensor, offset=t.offset + offset, ap=[t.ap[0]] + dims)


def _build_nc():
    nc = bacc.Bacc()
    z_d = nc.declare_dram_parameter("z", [R + PAD_ROWS, 16], FP, isOutput=False)
    c_d = nc.declare_dram_parameter("consts", [128, C_TOTAL], FP, isOutput=False)
    w_d = nc.declare_dram_parameter("wmat", [128, 6 * 128], BF, isOutput=False)
    o_d = nc.declare_dram_parameter("out", [R + PAD_ROWS, 16], FP, isOutput=True)

    with tile.TileContext(nc) as tc:
        with (
            tc.tile_pool(name="consts", bufs=1) as cp,
            tc.tile_pool(name="nat", bufs=1) as natp,
            tc.tile_pool(name="sb", bufs=1) as sbp,
            tc.tile_pool(name="ps", bufs=1, space="PSUM") as psp,
        ):
            consts = cp.tile([128, C_TOTAL], FP)
            nc.sync.dma_start(out=consts, in_=c_d[:, :])
            wmat = cp.tile([128, 6 * 128], BF)
            nc.sync.dma_start(out=wmat, in_=w_d[:, :])
            identbf = wmat[:, 5 * 128:6 * 128]
            lhsT = [wmat[:, k * 128:(k + 1) * 128] for k in range(5)]
            biases = [consts[:, C_BIAS + k:C_BIAS + k + 1] for k in range(7)]

            # warm-up scalar/vector vector clocks on the const DMAs
            wu1 = sbp.tile([128, 1], FP, tag="wu")
            nc.scalar.copy(out=wu1, in_=biases[0])
            wu2 = sbp.tile([128, 1], FP, tag="wu")
            nc.vector.tensor_copy(out=wu2, in_=biases[0])

            natbfs = {}

            def load(m):
                if m >= MACROS:
                    return
                r0 = m * ROWS_PER_MACRO
                natbf = natp.tile([128, 2048], BF, tag="nat", bufs=4)
                nc.gpsimd.dma_start(
                    out=natbf.rearrange("p (c f) -> p c f", c=128, f=16),
                    in_=z_d[r0:r0 + ROWS_PER_MACRO, :].rearrange(
                        "(p c) f -> p c f", p=128, c=128
                    ),
                )
                natbfs[m] = natbf

            def fwdT_quarter(m, q, x0ps):
                for u in range(q * 4, q * 4 + 4):
                    nc.tensor.transpose(
                        x0ps[:, u * 128:(u + 1) * 128],
                        natbfs[m][:, u * 128:(u + 1) * 128],
                        identbf,
                    )

            def half_mms(lhsT_k, h_in, half, tag):
                hps = psp.tile([128, 1024], FP, tag=tag, bufs=1)
                for n in range(2):
                    src = h_in[:, half * 1024 + n * 512:half * 1024 + (n + 1) * 512]
                    nc.tensor.matmul(hps[:, n * 512:(n + 1) * 512],
                                     lhsT_k, src, start=True, stop=True)
                return hps

            def half_prelu(hps, k, hb, half):
                nc.scalar.activation(
                    out=hb[:, half * 1024:(half + 1) * 1024], in_=hps,
                    func=mybir.ActivationFunctionType.Prelu,
                    bias=biases[k], scale=1.0, alpha=0.01,
                )

            def backT_quarter(eT, bT, eb, q):
                for u in range(q * 2, q * 2 + 2):
                    nc.tensor.transpose(
                        eT[:, u * 128:(u + 1) * 128],
                        eb[:, u * 128:(u + 1) * 128],
                        identbf,
                    )
                    nc.tensor.transpose(
                        bT[:, u * 128:(u + 1) * 128],
                        eb[:, 1024 + u * 128:1024 + (u + 1) * 128],
                        identbf,
                    )

            def combine_half(eT, bT, natbf, half):
                e_ap = _ap(eT, 64 * half, [[128, 8], [8, 8], [1, 8]])
                b_ap = _ap(bT, 64 * half, [[128, 8], [8, 8], [1, 8]])
                zr_ap = _ap(natbf, half * 1024 + 8, [[128, 8], [16, 8], [1, 8]])
                tmp = sbp.tile([128, 1024], BF, tag="tmp", bufs=2)
                tmp_ap = _ap(tmp, 0, [[128, 8], [8, 8], [1, 8]])
                nc.vector.tensor_mul(out=tmp_ap, in0=e_ap, in1=zr_ap)
                nc.vector.tensor_add(out=zr_ap, in0=tmp_ap, in1=b_ap)

            def store(m):
                r0 = m * ROWS_PER_MACRO
                out_dma = nc.gpsimd.dma_start(
                    out=o_d[r0:r0 + ROWS_PER_MACRO, :].rearrange(
                        "(p c) f -> p c f", p=128, c=128
                    ),
                    in_=natbfs[m].rearrange("p (c f) -> p c f", c=128, f=16),
                )
                del natbfs[m]
                load(m + 3)
                if m >= MACROS - 4:
                    tail_dmas.append(out_dma)

            def layer1(m, x0):
                """L1 of macro m (fills PE during macro m-1's Exp window)."""
                hb = sbp.tile([128, 2048], BF, tag="h0", bufs=2)
                hA = half_mms(lhsT[0], x0, 0, "hA")
                hB = half_mms(lhsT[0], x0, 1, "hB")
                half_prelu(hA, 0, hb, 0)
                half_prelu(hB, 0, hb, 1)
                return hb

            # Software pipeline, skewed by one layer: iteration m runs layers
            # 2..5 of macro m (A/B halves ping-ponging PE against ACT), with
            # macro m+1's fwd transposes in the layer slots, its L1 in macro
            # m's Exp window, and macro m's backT/combine/store at the end.
            tail_dmas = []
            load(0)
            load(1)
            load(2)
            x0ps = psp.tile([128, 2048], BF, tag="x0ps", bufs=1)
            for q in range(4):
                fwdT_quarter(0, q, x0ps)
            x0 = sbp.tile([128, 2048], BF, tag="x0", bufs=2)
            nc.vector.tensor_copy(out=x0, in_=x0ps)
            h = layer1(0, x0)

            for m in range(MACROS):
                nxt = m + 1 < MACROS
                eb = sbp.tile([128, 2048], BF, tag="eb", bufs=2)
                if nxt:
                    x0ps = psp.tile([128, 2048], BF, tag="x0ps", bufs=1)
                for k in (1, 2, 3):
                    hb = sbp.tile([128, 2048], BF, tag=f"h{k}", bufs=2)
                    hA = half_mms(lhsT[k], h, 0, "hA")
                    hB = half_mms(lhsT[k], h, 1, "hB")
                    if nxt:
                        fwdT_quarter(m + 1, k - 1, x0ps)
                    half_prelu(hA, k, hb, 0)
                    half_prelu(hB, k, hb, 1)
                    h = hb
                # L5: packed layout. eps[0:64]=e(A-cols), eps[64:128]=e(B-cols)
                lhsT5e = lhsT[4][:, 0:64]
                lhsT5b = lhsT[4][:, 64:128]
                if nxt:
                    fwdT_quarter(m + 1, 3, x0ps)
                    x0 = sbp.tile([128, 2048], BF, tag="x0", bufs=2)
                    nc.vector.tensor_copy(out=x0, in_=x0ps)
                eps = psp.tile([128, 1024], FP, tag="hA", bufs=1)
                bps = psp.tile([128, 1024], FP, tag="hB", bufs=1)
                for half in range(2):
                    for n in range(2):
                        src = h[:, half * 1024 + n * 512:half * 1024 + (n + 1) * 512]
                        nc.tensor.matmul(
                            eps[64 * half:64 * half + 64, n * 512:(n + 1) * 512],
                            lhsT5e, src, start=True, stop=True)
                nc.scalar.activation(
                    out=eb[:, 0:1024], in_=eps,
                    func=mybir.ActivationFunctionType.Exp,
                    bias=biases[5], scale=1.0,
                )
                for half in range(2):
                    for n in range(2):
                        src = h[:, half * 1024 + n * 512:half * 1024 + (n + 1) * 512]
                        nc.tensor.matmul(
                            bps[64 * half:64 * half + 64, n * 512:(n + 1) * 512],
                            lhsT5b, src, start=True, stop=True)
                nc.vector.tensor_scalar_add(
                    out=eb[:, 1024:2048], in0=bps, scalar1=biases[6],
                )
                if nxt:
                    # macro m+1's L1 keeps PE busy through the Exp window
                    h = layer1(m + 1, x0)

                # ---- back transposes -> eT/bT (bf16 PSUM), combine, store
                eT = psp.tile([128, 1024], BF, tag="eT", bufs=1)
                bT = psp.tile([128, 1024], BF, tag="bT", bufs=1)
                for q in range(4):
                    backT_quarter(eT, bT, eb, q)
                combine_half(eT, bT, natbfs[m], 0)
                combine_half(eT, bT, natbfs[m], 1)
                store(m)

            flush = sbp.tile([128, 1], FP, tag="wu")
            fl = nc.vector.tensor_copy(out=flush, in_=biases[0])
            for dma in tail_dmas:
                _add_dep_helper(fl.ins, dma.ins, sync=True,
                                reason="drain tail out-DMAs before kernel end")

    nc.finalize()
    return nc


_NC_CACHE = None


def kernel(z, ws_logs, bs_logs, ws_b, bs_b):
    global _NC_CACHE, LAST_RESULTS
    z = np.asarray(z, np.float32)
    assert z.shape == (BATCH, 16)
    consts, wmat_bf = _build_consts(ws_logs, bs_logs, ws_b, bs_b)

    if _NC_CACHE is None:
        _NC_CACHE = _build_nc()
    nc = _NC_CACHE

    in_maps = []
    for s in STARTS:
        zp = np.zeros((R + PAD_ROWS, 16), np.float32)
        zp[:R] = z[s:s + R]
        in_maps.append({"z": zp, "consts": consts, "wmat": wmat_bf})
    trace = bool(os.environ.get("AFFINE_TRACE"))
    res = run_bass_kernel_spmd(nc, in_maps, core_ids=list(range(N_CORES)), trace=trace)
    LAST_RESULTS = res

    out = np.empty((BATCH, 16), np.float32)
    for c in range(N_CORES):
        out[STARTS[c]:STARTS[c] + R] = res.results[c]["out"][:R]
    return out


# revision 10
# speedup vs baseline: 1.1054x; 1.1054x over previous
"""AffineCoupling TRN2 kernel (v6).

Computes, for z [4_000_000, 16] fp32:
    zl = z[:, :8]; zr = z[:, 8:]
    log_s = MLP_logs(zl); b = MLP_b(zl)        (5 layers, LeakyReLU(0.01) between)
    out = concat([zl, yr]), yr = exp(log_s) * zr + b

Strategy (pure data parallel over 8 NeuronCores, ~508k rows each):
 - Contiguous DMA: core slice split into 31 macros of 16384 rows. natbf
   [128, 2048] bf16 holds 128 rows/partition (nat[p, c*16+f] = row p*128+c),
   loaded by ONE SWDGE cast-DMA (fp32 HBM -> bf16 SBUF, 8KB contiguous HBM
   per partition) and stored back by one SWDGE cast-DMA (bf16 -> fp32).
   The whole pipeline is bf16 (zl passthrough in bf16: ~1.3e-3 rel err,
   tolerance is 2e-2).
 - fwdT: 16 PE transpose-mode ops [128,128] -> x0ps bf16 PSUM (feature-major
   X layout: partition g*16+f, 8 groups of 16 feats); DVE 2x copy -> x0.
 - MLP: both branches fused in 16-wide groups (block-diagonal bf16 lhsT,
   same wmat as before); per layer 4 MMs N=512 -> h fp32 PSUM [128, 2048],
   one ACT Prelu (bias via per-partition operand) -> bf16 SBUF.
 - L5 -> hp5 [128, 2048] fp32 (e at partitions 0:64 as g*8+o, b at 64:128);
   ACT Exp (+bias) and DVE tensor_scalar_add assemble eb bf16.
 - backT: 16 transpose-mode ops -> ebT bf16 PSUM; combine in place:
   natbf_zr = e*zr + b via 2x (mul into tmp, add back), u-batched APs.
 - PSUM: x0ps(2) + h(4) + ebT(2) = 8 banks, single-buffered per tag;
   cross-macro overlap comes from fwdT/backT of adjacent macros.
"""
import os
import sys

sys.path.insert(0, "/opt/trn_rl_repo")
if "/root/.axon_site/_ro/trn_rl_repo" not in sys.path:
    sys.path.append("/root/.axon_site/_ro/trn_rl_repo")

import numpy as np

import concourse.bacc as bacc
import concourse.bass as bass
import concourse.tile as tile
from concourse import mybir
from concourse.bass import _add_dep_helper
from concourse.bass_utils import run_bass_kernel_spmd

FP = mybir.dt.float32
BF = mybir.dt.bfloat16

N_CORES = 8
BATCH = 4_000_000
ROWS_PER_MACRO = 16_384            # [128, 2048] bf16 nat tile, 128 rows/part
MACROS = 31
R = ROWS_PER_MACRO * MACROS        # 507,904 rows per core
PAD_ROWS = ROWS_PER_MACRO
CHUNKS = 4                         # 4096 rows each

STEP = 498_688
STARTS = [c * STEP for c in range(N_CORES - 1)] + [BATCH - R]

C_BIAS = 128
C_TOTAL = 135

LAST_RESULTS = None


def _build_consts(ws_logs, bs_logs, ws_b, bs_b):
    import ml_dtypes

    ws_logs = [np.asarray(w, np.float32) for w in ws_logs]
    bs_logs = [np.asarray(b, np.float32) for b in bs_logs]
    ws_b = [np.asarray(w, np.float32) for w in ws_b]
    bs_b = [np.asarray(b, np.float32) for b in bs_b]

    consts = np.zeros((128, C_TOTAL), np.float32)
    consts[:, 0:128] = np.eye(128, dtype=np.float32)
    for k in range(4):
        cat = np.concatenate([bs_logs[k], bs_b[k]])    # [16]
        consts[:, C_BIAS + k] = np.tile(cat, 8)
    consts[:, C_BIAS + 4] = np.concatenate(
        [np.tile(bs_logs[4], 8), np.tile(bs_b[4], 8)]
    )
    consts[:, C_BIAS + 5] = np.tile(bs_logs[4], 16)
    consts[:, C_BIAS + 6] = np.tile(bs_b[4], 16)

    # bf16 stationary matrices, lhsT k at cols [k*128, (k+1)*128)
    wmat = np.zeros((128, 5 * 128), np.float32)
    w1cat = np.vstack([ws_logs[0], ws_b[0]])           # [16, 8]
    for g in range(8):
        wmat[g * 16:g * 16 + 8, g * 16:(g + 1) * 16] = w1cat.T
    for k in (1, 2, 3):
        wk = np.zeros((16, 16), np.float32)
        wk[0:8, 0:8] = ws_logs[k]
        wk[8:16, 8:16] = ws_b[k]
        for g in range(8):
            wmat[g * 16:(g + 1) * 16, k * 128 + g * 16:k * 128 + (g + 1) * 16] = wk.T
    for g in range(8):
        wmat[g * 16:g * 16 + 8, 4 * 128 + g * 8:4 * 128 + (g + 1) * 8] = ws_logs[4].T
        wmat[g * 16 + 8:(g + 1) * 16,
             4 * 128 + 64 + g * 8:4 * 128 + 64 + (g + 1) * 8] = ws_b[4].T
    wmat = np.concatenate([wmat, np.eye(128, dtype=np.float32)], axis=1)
    wmat_bf = wmat.astype(ml_dtypes.bfloat16)
    return consts, wmat_bf


def _ap(t, offset, dims):
    return bass.AP(tensor=t.tensor, offset=t.offset + offset, ap=[t.ap[0]] + dims)


def _build_nc():
    nc = bacc.Bacc()
    z_d = nc.declare_dram_parameter("z", [R + PAD_ROWS, 16], FP, isOutput=False)
    c_d = nc.declare_dram_parameter("consts", [128, C_TOTAL], FP, isOutput=False)
    w_d = nc.declare_dram_parameter("wmat", [128, 6 * 128], BF, isOutput=False)
    o_d = nc.declare_dram_parameter("out", [R + PAD_ROWS, 16], FP, isOutput=True)

    with tile.TileContext(nc) as tc:
        with (
            tc.tile_pool(name="consts", bufs=1) as cp,
            tc.tile_pool(name="nat", bufs=1) as natp,
            tc.tile_pool(name="sb", bufs=1) as sbp,
            tc.tile_pool(name="ps", bufs=1, space="PSUM") as psp,
        ):
            consts = cp.tile([128, C_TOTAL], FP)
            nc.sync.dma_start(out=consts, in_=c_d[:, :])
            wmat = cp.tile([128, 6 * 128], BF)
            nc.sync.dma_start(out=wmat, in_=w_d[:, :])
            identbf = wmat[:, 5 * 128:6 * 128]
            lhsT = [wmat[:, k * 128:(k + 1) * 128] for k in range(5)]
            biases = [consts[:, C_BIAS + k:C_BIAS + k + 1] for k in range(7)]

            # warm-up scalar/vector vector clocks on the const DMAs
            wu1 = sbp.tile([128, 1], FP, tag="wu")
            nc.scalar.copy(out=wu1, in_=biases[0])
            wu2 = sbp.tile([128, 1], FP, tag="wu")
            nc.vector.tensor_copy(out=wu2, in_=biases[0])

            natbfs = {}

            def load(m):
                if m >= MACROS:
                    return
                r0 = m * ROWS_PER_MACRO
                natbf = natp.tile([128, 2048], BF, tag="nat", bufs=4)
                nc.gpsimd.dma_start(
                    out=natbf.rearrange("p (c f) -> p c f", c=128, f=16),
                    in_=z_d[r0:r0 + ROWS_PER_MACRO, :].rearrange(
                        "(p c) f -> p c f", p=128, c=128
                    ),
                )
                natbfs[m] = natbf

            def fwdT_quarter(m, q, x0ps):
                for u in range(q * 4, q * 4 + 4):
                    nc.tensor.transpose(
                        x0ps[:, u * 128:(u + 1) * 128],
                        natbfs[m][:, u * 128:(u + 1) * 128],
                        identbf,
                    )

            def half_mms(lhsT_k, h_in, half, tag):
                hps = psp.tile([128, 1024], FP, tag=tag, bufs=1)
                for n in range(2):
                    src = h_in[:, half * 1024 + n * 512:half * 1024 + (n + 1) * 512]
                    nc.tensor.matmul(hps[:, n * 512:(n + 1) * 512],
                                     lhsT_k, src, start=True, stop=True)
                return hps

            def half_prelu(hps, k, hb, half):
                nc.scalar.activation(
                    out=hb[:, half * 1024:(half + 1) * 1024], in_=hps,
                    func=mybir.ActivationFunctionType.Prelu,
                    bias=biases[k], scale=1.0, alpha=0.01,
                )

            def backT_quarter(eT, bT, eb, q):
                for u in range(q * 2, q * 2 + 2):
                    nc.tensor.transpose(
                        eT[:, u * 128:(u + 1) * 128],
                        eb[:, u * 128:(u + 1) * 128],
                        identbf,
                    )
                    nc.tensor.transpose(
                        bT[:, u * 128:(u + 1) * 128],
                        eb[:, 1024 + u * 128:1024 + (u + 1) * 128],
                        identbf,
                    )

            def combine_half(eT, bT, natbf, half):
                e_ap = _ap(eT, 64 * half, [[128, 8], [8, 8], [1, 8]])
                b_ap = _ap(bT, 64 * half, [[128, 8], [8, 8], [1, 8]])
                zr_ap = _ap(natbf, half * 1024 + 8, [[128, 8], [16, 8], [1, 8]])
                tmp = sbp.tile([128, 1024], BF, tag="tmp", bufs=2)
                tmp_ap = _ap(tmp, 0, [[128, 8], [8, 8], [1, 8]])
                nc.vector.tensor_mul(out=tmp_ap, in0=e_ap, in1=zr_ap)
                nc.vector.tensor_add(out=zr_ap, in0=tmp_ap, in1=b_ap)

            def store(m):
                r0 = m * ROWS_PER_MACRO
                out_dma = nc.gpsimd.dma_start(
                    out=o_d[r0:r0 + ROWS_PER_MACRO, :].rearrange(
                        "(p c) f -> p c f", p=128, c=128
                    ),
                    in_=natbfs[m].rearrange("p (c f) -> p c f", c=128, f=16),
                )
                del natbfs[m]
                load(m + 3)
                if m >= MACROS - 4:
                    tail_dmas.append(out_dma)

            def layer1(m, x0):
                """L1 of macro m (fills PE during macro m-1's Exp window)."""
                hb = sbp.tile([128, 2048], BF, tag="h0", bufs=2)
                hA = half_mms(lhsT[0], x0, 0, "hA")
                hB = half_mms(lhsT[0], x0, 1, "hB")
                half_prelu(hA, 0, hb, 0)
                half_prelu(hB, 0, hb, 1)
                return hb

            # Software pipeline, skewed by one layer: iteration m runs layers
            # 2..5 of macro m (A/B halves ping-ponging PE against ACT), with
            # macro m+1's fwd transposes in the layer slots, its L1 in macro
            # m's Exp window, and macro m's backT/combine/store at the end.
            tail_dmas = []
            load(0)
            load(1)
            load(2)
            x0ps = psp.tile([128, 2048], BF, tag="x0ps", bufs=1)
            for q in range(4):
                fwdT_quarter(0, q, x0ps)
            x0 = sbp.tile([128, 2048], BF, tag="x0", bufs=2)
            nc.vector.tensor_copy(out=x0, in_=x0ps)
            h = layer1(0, x0)

            for m in range(MACROS):
                nxt = m + 1 < MACROS
                eb = sbp.tile([128, 2048], BF, tag="eb", bufs=2)
                if nxt:
                    x0ps = psp.tile([128, 2048], BF, tag="x0ps", bufs=1)
                for k in (1, 2, 3):
                    hb = sbp.tile([128, 2048], BF, tag=f"h{k}", bufs=2)
                    hA = half_mms(lhsT[k], h, 0, "hA")
                    hB = half_mms(lhsT[k], h, 1, "hB")
                    if nxt:
                        fwdT_quarter(m + 1, k - 1, x0ps)
                    half_prelu(hA, k, hb, 0)
                    half_prelu(hB, k, hb, 1)
                    h = hb
                # L5: packed layout. eps[0:64]=e(A-cols), eps[64:128]=e(B-cols)
                lhsT5e = lhsT[4][:, 0:64]
                lhsT5b = lhsT[4][:, 64:128]
                eps = psp.tile([128, 1024], FP, tag="hA", bufs=1)
                bps = psp.tile([128, 1024], FP, tag="hB", bufs=1)
                for half in range(2):
                    for n in range(2):
                        src = h[:, half * 1024 + n * 512:half * 1024 + (n + 1) * 512]
                        nc.tensor.matmul(
                            eps[64 * half:64 * half + 64, n * 512:(n + 1) * 512],
                            lhsT5e, src, start=True, stop=True)
                        nc.tensor.matmul(
                            bps[64 * half:64 * half + 64, n * 512:(n + 1) * 512],
                            lhsT5b, src, start=True, stop=True)
                if nxt:
                    fwdT_quarter(m + 1, 3, x0ps)
                    x0 = sbp.tile([128, 2048], BF, tag="x0", bufs=2)
                    nc.vector.tensor_copy(out=x0, in_=x0ps)
                nc.scalar.activation(
                    out=eb[:, 0:1024], in_=eps,
                    func=mybir.ActivationFunctionType.Exp,
                    bias=biases[5], scale=1.0,
                )
                nc.vector.tensor_scalar_add(
                    out=eb[:, 1024:2048], in0=bps, scalar1=biases[6],
                )
                if nxt:
                    # macro m+1's L1 keeps PE busy through the Exp window
                    h = layer1(m + 1, x0)

                # ---- back transposes -> eT/bT (bf16 PSUM), combine, store
                eT = psp.tile([128, 1024], BF, tag="eT", bufs=1)
                bT = psp.tile([128, 1024], BF, tag="bT", bufs=1)
                for q in range(4):
                    backT_quarter(eT, bT, eb, q)
                combine_half(eT, bT, natbfs[m], 0)
                combine_half(eT, bT, natbfs[m], 1)
                store(m)

            flush = sbp.tile([128, 1], FP, tag="wu")
            fl = nc.vector.tensor_copy(out=flush, in_=biases[0])
            for dma in tail_dmas:
                _add_dep_helper(fl.ins, dma.ins, sync=True,
                                reason="drain tail out-DMAs before kernel end")

    nc.finalize()
    return nc


_NC_CACHE = None


def kernel(z, ws_logs, bs_logs, ws_b, bs_b):
    global _NC_CACHE, LAST_RESULTS
    z = np.asarray(z, np.float32)
    assert z.shape == (BATCH, 16)
    consts, wmat_bf = _build_consts(ws_logs, bs_logs, ws_b, bs_b)

    if _NC_CACHE is None:
        _NC_CACHE = _build_nc()
    nc = _NC_CACHE

    in_maps = []
    for s in STARTS:
        zp = np.zeros((R + PAD_ROWS, 16), np.float32)
        zp[:R] = z[s:s + R]
        in_maps.append({"z": zp, "consts": consts, "wmat": wmat_bf})
    trace = bool(os.environ.get("AFFINE_TRACE"))
    res = run_bass_kernel_spmd(nc, in_maps, core_ids=list(range(N_CORES)), trace=trace)
    LAST_RESULTS = res

    out = np.empty((BATCH, 16), np.float32)
    for c in range(N_CORES):
        out[STARTS[c]:STARTS[c] + R] = res.results[c]["out"][:R]
    return out
